# revision 25
# baseline (speedup 1.0000x reference)
"""Trainium2 Bass kernel for nn_LossConsistenciaMorfologicaCompuesta.

Composite morphological-consistency loss:
  for k in (3,5,7): Dice(pred, dilate_k(teacher)) + Dice(pred, erode_k(teacher)),
  total/3, cv2-ellipse structuring elements, Dice reduced over (batch, pixels).

Strategy (8 NeuronCores, data-parallel over batch B=16 -> 2 images/core):
  - Slab layout: one 1024x1024 image in SBUF as [128 partitions, 8 rows(+halo),
    1024 cols] fp16 (DVE 2x mode). Vertical shifts are free-dim row offsets;
    halo rows come from tiny partition-shifted SBUF->SBUF DMAs (replicate at
    image edges -- exact for flat morphology).
  - Ellipse decomposition (exact, 13 DVE folds per side):
      h1   = hmax3(t);  m3 = max(h1, t up1, t dn1)          (ellipse3 = plus)
      m5   = max(m3 l1, r1, up1, dn1)                        (ellipse5 = diamond2)
      hm2  = max(t l2, t r2)  (halo rows via DMA)
      m7   = max(hm2 up2, hm2 dn2, m5 l1, r1, up1, dn1)     (ellipse7)
    erosion mirrored with min. DVE is the only engine with elementwise
    max/min on this target, so everything else rides other engines:
  - sum(p*m) on PE: "diagonal" matmuls (weights = 128-col blocks of p,
    moving = m blocks) accumulated into a [128,128] PSUM region whose
    diagonal holds the inter partials; extracted once at the end with a
    tensor_tensor_reduce against an identity matrix.
  - sum(m) on PE: ones-weight matmuls into a [1,384] PSUM region sharing
    the same bank as the diag tile (PSUM is bank-granular, 8 banks).
  - sum(p) rides the f32->f16 cast on ACT as accum_out.
  - Each core writes 20 partial sums; the host combines into the scalar.
"""

import numpy as np

B, C_IN, H, W = 16, 1, 1024, 1024
NCORES = 8
BPC = B // NCORES      # images per core
P = 128                # SBUF partitions
R = H // P             # 8 slab rows per partition
EPS = 1e-7
CH = 2                 # slab rows per f32 staging chunk
NSUM = 18              # sums cols: 8 p-cast + 6 inter + 4 tail-msum
NOUT = 30              # 18 sums + 12 msum scalars

_CACHE = {}


def build_nc():
    import concourse.bacc as bacc
    import concourse.mybir as mybir
    import concourse.tile as tile

    f32 = mybir.dt.float32
    f16 = mybir.dt.float16
    MAX = mybir.AluOpType.max
    MIN = mybir.AluOpType.min
    ADD = mybir.AluOpType.add
    MULT = mybir.AluOpType.mult
    COPY = mybir.ActivationFunctionType.Copy

    C = W
    n_img = BPC
    TROWS = R + 4   # t/hm2: 2 halo rows above+below
    MROWS = R + 2   # m3/m5: 1 halo row above+below
    MC = C + 2      # m3/m5: 1 pad col each side (buf col = img col + 1)

    nc = bacc.Bacc("TRN2", target_bir_lowering=False)
    t_dram = nc.dram_tensor("teacher", [n_img, R * P, C], f32, kind="ExternalInput")
    p_dram = nc.dram_tensor("pred", [n_img, R * P, C], f32, kind="ExternalInput")
    i_dram = nc.dram_tensor("ident", [P, P], f32, kind="ExternalInput")
    out_dram = nc.dram_tensor("partials", [1, NOUT], f32, kind="ExternalOutput")

    with tile.TileContext(nc) as tc:
        with (
            tc.tile_pool(name="stage", bufs=2) as stage_pool,
            tc.tile_pool(name="img", bufs=2) as img_pool,
            tc.tile_pool(name="imgp", bufs=1) as imgp_pool,
            tc.tile_pool(name="morph2", bufs=2) as morph2_pool,
            tc.tile_pool(name="morph", bufs=1) as morph_pool,
            tc.tile_pool(name="small", bufs=1) as small_pool,
            tc.tile_pool(name="psum", bufs=1, space="PSUM") as psum_pool,
        ):
            sums = small_pool.tile([P, NSUM], f32, tag="sums")
            ones16 = small_pool.tile([P, 1], f16, tag="ones16")
            ones32 = small_pool.tile([P, 1], f32, tag="ones32")
            ident = small_pool.tile([P, P], f32, tag="ident")
            scr = small_pool.tile([P, P], f32, tag="scr")
            scr512 = small_pool.tile([1, 512], f32, tag="scr512")
            scr2 = small_pool.tile([P, P], f32, tag="scr2")
            msums1 = small_pool.tile([1, 12], f32, tag="msums1")
            outsb = small_pool.tile([1, NOUT], f32, tag="outsb")
            nc.vector.memset(sums[:], 0.0)
            nc.vector.memset(ones16[:], 1.0)
            nc.vector.memset(ones32[:], 1.0)
            nc.sync.dma_start(ident[:], i_dram[:])

            m5 = morph_pool.tile([P, MROWS, MC], f16, tag="m5")
            hm2 = morph_pool.tile([P, TROWS, C], f16, tag="hm2")

            # one PSUM bank per quantity holding the [128,128] diag tile,
            # plus a single shared [1,512] bank for the msum chains
            ps_q = [psum_pool.tile([P, P], f32, tag=f"psq{q}", name=f"psq{q}")
                    for q in range(6)]
            ps_ms = [psum_pool.tile([1, 512], f32, tag="psms0", name="psms0"),
                     psum_pool.tile([1, 512], f32, tag="psms1", name="psms1")]
            diag_cnt = [0] * 6
            iblk = [(c0, 128) for c0 in range(0, C, 128)]
            DIAG_TOT = n_img * R * len(iblk)

            def halo1(m):
                """1-row top/bottom halo fill (replicate at image edges)."""
                nc.sync.dma_start(m[1:P, 0:1, :], m[0:P - 1, R:R + 1, :])
                nc.sync.dma_start(m[0:P - 1, MROWS - 1:MROWS, :], m[1:P, 1:2, :])
                nc.sync.dma_start(m[0:1, 0:1, :], m[0:1, 1:2, :])
                nc.sync.dma_start(m[P - 1:P, MROWS - 1:MROWS, :],
                                  m[P - 1:P, MROWS - 2:MROWS - 1, :])

            def halo2(m):
                """2-row top/bottom halo fill for a TROWS buffer."""
                nc.sync.dma_start(m[1:P, 0:2, :], m[0:P - 1, R:R + 2, :])
                nc.sync.dma_start(m[0:P - 1, TROWS - 2:TROWS, :], m[1:P, 2:4, :])
                for hr in (0, 1):
                    nc.sync.dma_start(m[0:1, hr:hr + 1, :], m[0:1, 2:3, :])
                for hr in (TROWS - 2, TROWS - 1):
                    nc.sync.dma_start(m[P - 1:P, hr:hr + 1, :],
                                      m[P - 1:P, TROWS - 3:TROWS - 2, :])

            def pe_sums(q, img, p_t, buf, rofs, cofs, r0=0, r1=R,
                        msum_pe=True):
                """sum(p*m) on PE via diagonal-accumulate matmuls; sum(m) on
                PE via a ones-matmul chain through the shared msum bank,
                read out by ACT into msums1 before the next chain reuses it."""
                for r in range(r0, r1):
                    if msum_pe:
                        for c0 in (0, 512):
                            nc.tensor.matmul(
                                ps_ms[q % 2][:, 0:512], ones16[:],
                                buf[:, rofs + r, cofs + c0:cofs + c0 + 512],
                                start=(r == 0 and c0 == 0),
                                stop=(r == R - 1 and c0 == 512))
                    for (c0, cw) in iblk:
                        nc.tensor.matmul(
                            ps_q[q][0:cw, 0:cw], p_t[:, r, c0:c0 + cw],
                            buf[:, rofs + r, cofs + c0:cofs + c0 + cw],
                            start=(diag_cnt[q] == 0),
                            stop=(diag_cnt[q] == DIAG_TOT - 1))
                        diag_cnt[q] += 1
                if r1 < R:
                    return
                if msum_pe:
                    nc.scalar.activation(scr512[:], ps_ms[q % 2][:], COPY,
                                         accum_out=msums1[:, img * 6 + q:
                                                          img * 6 + q + 1])
                if diag_cnt[q] == DIAG_TOT:
                    # chain complete: extract the PSUM diagonal now, off the
                    # critical tail
                    nc.scalar.activation(scr[:], ps_q[q][:], COPY)
                    nc.vector.tensor_tensor(scr2[:], scr[:], ident[:], op=MULT)
                    nc.vector.tensor_reduce(sums[:, 8 + q:9 + q], scr2[:],
                                            axis=mybir.AxisListType.X, op=ADD)

            for img in range(n_img):
                t_view = t_dram[img].rearrange("(p r) w -> p r w", p=P)
                p_view = p_dram[img].rearrange("(p r) w -> p r w", p=P)
                t = img_pool.tile([P, TROWS, C], f16, tag="t", name="t")
                p_t = imgp_pool.tile([P, R, C], f16, tag="p", name="p")
                for (ra, rb) in ((0, 2), (2, 4), (4, 6), (6, 8)):
                    st = stage_pool.tile([P, CH, C], f32, tag="stage", name="stage")
                    nc.sync.dma_start(st[:], t_view[:, ra:rb, :])
                    if img == 0 and ra >= R // 2:
                        # first image: DVE (idle at startup) casts the second
                        # half of t so it isn't bound by the serial ACT casts
                        nc.vector.tensor_copy(t[:, 2 + ra:2 + rb, :], st[:])
                    else:
                        nc.scalar.activation(t[:, 2 + ra:2 + rb, :], st[:],
                                             COPY)
                halo2(t)
                for r0 in range(0, R, CH):
                    st = stage_pool.tile([P, CH, C], f32, tag="stage", name="stage")
                    nc.sync.dma_start(st[:], p_view[:, r0:r0 + CH, :])
                    nc.scalar.activation(p_t[:, r0:r0 + CH, :], st[:], COPY,
                                         accum_out=sums[:, img * 4 + r0 // CH:
                                                        img * 4 + r0 // CH + 1])

                for is_dil, base_q in ((True, 0), (False, 3)):
                    OP = MAX if is_dil else MIN
                    fill = -1e4 if is_dil else 1e4
                    V = nc.vector
                    m3 = morph2_pool.tile([P, MROWS, MC], f16, tag="m3",
                                          name="m3")
                    m7 = morph2_pool.tile([P, R, C], f16, tag="m7", name="m7")
                    first = img == 0 and is_dil
                    rsp = (((0, 2), (2, 4), (4, 6), (6, 8)) if first
                           else ((0, 8),))

                    V.memset(m3[:, :, 0:1], fill)
                    V.memset(m3[:, :, MC - 1:MC], fill)
                    V.memset(m5[:, :, 0:1], fill)
                    V.memset(m5[:, :, MC - 1:MC], fill)

                    def emit_h1(OP=OP, rsp=rsp):
                        # hmax3(t) built directly into m3 (no h1 buffer):
                        # m3 = op(t l1, t r1); then op= t center
                        for (ra, rb) in rsp:
                            V.tensor_tensor(m3[:, 1 + ra:1 + rb, 2:C],
                                            t[:, 2 + ra:2 + rb, 0:C - 2],
                                            t[:, 2 + ra:2 + rb, 2:C], op=OP)
                        V.tensor_tensor(m3[:, 1:9, 1:2], t[:, 2:10, 1:2],
                                        t[:, 2:10, 1:2], op=OP)
                        V.tensor_tensor(m3[:, 1:9, C:C + 1],
                                        t[:, 2:10, C - 2:C - 1],
                                        t[:, 2:10, C - 2:C - 1], op=OP)
                        for (ra, rb) in rsp:
                            V.tensor_tensor(m3[:, 1 + ra:1 + rb, 1:C + 1],
                                            m3[:, 1 + ra:1 + rb, 1:C + 1],
                                            t[:, 2 + ra:2 + rb, :], op=OP)

                    def emit_hm2(OP=OP, rsp=rsp):
                        # hm2 = op(t l2, t r2); halo rows via DMA. Emitted
                        # early so the halo DMAs hide behind later DVE work.
                        for (ra, rb) in rsp:
                            V.tensor_tensor(hm2[:, 2 + ra:2 + rb, 2:C - 2],
                                            t[:, 2 + ra:2 + rb, 0:C - 4],
                                            t[:, 2 + ra:2 + rb, 4:C], op=OP)
                        V.tensor_tensor(hm2[:, 2:10, 0:2], t[:, 2:10, 2:4],
                                        t[:, 2:10, 2:4], op=OP)
                        V.tensor_tensor(hm2[:, 2:10, C - 2:C],
                                        t[:, 2:10, C - 4:C - 2],
                                        t[:, 2:10, C - 4:C - 2], op=OP)
                        halo2(hm2)

                    if first:
                        emit_h1()
                        emit_hm2()
                    else:
                        emit_hm2()
                        emit_h1()

                    # ---- m3 = op(hmax3, t up1, t dn1) ----
                    V.tensor_tensor(m3[:, 1:9, 1:C + 1], m3[:, 1:9, 1:C + 1],
                                    t[:, 1:9, :], op=OP)
                    V.tensor_tensor(m3[:, 1:9, 1:C + 1], m3[:, 1:9, 1:C + 1],
                                    t[:, 3:11, :], op=OP)
                    halo1(m3)
                    pe_sums(base_q + 0, img, p_t, m3, 1, 1)

                    # ---- m7 init = op(hm2 up2, dn2) ----
                    V.tensor_tensor(m7[:], hm2[:, 0:8, :], hm2[:, 4:12, :], op=OP)

                    # ---- m5 = op(m3 l1, r1, up1, dn1) ----
                    V.tensor_tensor(m5[:, 1:9, 1:C + 1], m3[:, 1:9, 0:C],
                                    m3[:, 1:9, 2:C + 2], op=OP)
                    V.tensor_tensor(m5[:, 1:9, 1:C + 1], m5[:, 1:9, 1:C + 1],
                                    m3[:, 0:8, 1:C + 1], op=OP)
                    V.tensor_tensor(m5[:, 1:9, 1:C + 1], m5[:, 1:9, 1:C + 1],
                                    m3[:, 2:10, 1:C + 1], op=OP)
                    halo1(m5)
                    pe_sums(base_q + 1, img, p_t, m5, 1, 1)

                    # ---- m7 folds: op(m7, m5 l1, r1, up1, dn1) ----
                    # Last side: fully row-quartered so PE/ACT consumption
                    # pipelines with the folds and the tail shrinks.
                    last = img == n_img - 1 and not is_dil
                    if last:
                        for qi, (ra, rb) in enumerate(
                                ((0, 2), (2, 4), (4, 6), (6, 8))):
                            V.tensor_tensor(m7[:, ra:rb, :], m7[:, ra:rb, :],
                                            m5[:, 1 + ra:1 + rb, 0:C], op=OP)
                            V.tensor_tensor(m7[:, ra:rb, :], m7[:, ra:rb, :],
                                            m5[:, 1 + ra:1 + rb, 2:C + 2],
                                            op=OP)
                            V.tensor_tensor(m7[:, ra:rb, :], m7[:, ra:rb, :],
                                            m5[:, ra:rb, 1:C + 1], op=OP)
                            V.tensor_tensor(m7[:, ra:rb, :], m7[:, ra:rb, :],
                                            m5[:, 2 + ra:2 + rb, 1:C + 1],
                                            op=OP)
                            pe_sums(base_q + 2, img, p_t, m7, 0, 0,
                                    r0=ra, r1=rb, msum_pe=False)
                            nc.scalar.activation(
                                m7[:, ra:rb, :], m7[:, ra:rb, :], COPY,
                                accum_out=sums[:, 14 + qi:15 + qi])
                    else:
                        V.tensor_tensor(m7[:], m7[:], m5[:, 1:9, 0:C], op=OP)
                        V.tensor_tensor(m7[:], m7[:], m5[:, 1:9, 2:C + 2], op=OP)
                        V.tensor_tensor(m7[:], m7[:], m5[:, 0:8, 1:C + 1], op=OP)
                        V.tensor_tensor(m7[:], m7[:], m5[:, 2:10, 1:C + 1], op=OP)
                        pe_sums(base_q + 2, img, p_t, m7, 0, 0)

            # ---- epilogue ----
            nc.tensor.matmul(ps_ms[0][0:1, 0:NSUM], ones32[:], sums[:],
                             start=True, stop=True)
            nc.scalar.activation(outsb[:, 0:NSUM], ps_ms[0][0:1, 0:NSUM], COPY)
            nc.scalar.activation(outsb[:, NSUM:NSUM + 12], msums1[:], COPY)
            nc.sync.dma_start(out_dram[:], outsb[:])

    nc.compile()
    return nc


def combine_partials(partials):
    """partials: [ncores, 26] float32 -> scalar loss (mirrors reference)."""
    partials = np.asarray(partials, dtype=np.float64)
    p_sum = partials[:, 0:8].sum()
    total = 0.0
    for q in range(6):
        inter = partials[:, 8 + q].sum()
        msum = 0.0
        for img in range(BPC):
            if img == BPC - 1 and q == 5:
                msum += partials[:, 14:18].sum()
            else:
                msum += partials[:, 18 + img * 6 + q].sum()
        card = p_sum + msum
        score = 2.0 * inter / max(card, EPS)
        total += (1.0 - score) * (1.0 if msum > 0 else 0.0)
    return np.float32(total / 3.0)


def kernel(pred_student_prob, teacher_prob):
    from concourse.bass_utils import run_bass_kernel_spmd

    if "nc" not in _CACHE:
        _CACHE["nc"] = build_nc()
    nc = _CACHE["nc"]

    pred = np.ascontiguousarray(pred_student_prob.reshape(B, H, W), dtype=np.float32)
    teach = np.ascontiguousarray(teacher_prob.reshape(B, H, W), dtype=np.float32)
    ident = np.eye(P, dtype=np.float32)
    in_maps = []
    for c in range(NCORES):
        sl = slice(c * BPC, (c + 1) * BPC)
        in_maps.append({
            "teacher": np.ascontiguousarray(teach[sl]),
            "pred": np.ascontiguousarray(pred[sl]),
            "ident": ident,
        })
    res = run_bass_kernel_spmd(nc, in_maps, core_ids=list(range(NCORES)))
    partials = np.stack([res.results[c]["partials"][0] for c in range(NCORES)])
    return combine_partials(partials)


# revision 27
# speedup vs baseline: 1.0067x; 1.0067x over previous
"""Trainium2 Bass kernel for nn_LossConsistenciaMorfologicaCompuesta.

Composite morphological-consistency loss:
  for k in (3,5,7): Dice(pred, dilate_k(teacher)) + Dice(pred, erode_k(teacher)),
  total/3, cv2-ellipse structuring elements, Dice reduced over (batch, pixels).

Strategy (8 NeuronCores, data-parallel over batch B=16 -> 2 images/core):
  - Slab layout: one 1024x1024 image in SBUF as [128 partitions, 8 rows(+halo),
    1024 cols] fp16 (DVE 2x mode). Vertical shifts are free-dim row offsets;
    halo rows come from tiny partition-shifted SBUF->SBUF DMAs (replicate at
    image edges -- exact for flat morphology).
  - Ellipse decomposition (exact, 13 DVE folds per side, built in place):
      m3  = max(t l1, r1, center, up1, dn1)                  (ellipse3 = plus)
      m5  = max(m3 l1, r1, up1, dn1)                         (ellipse5 = diamond2)
      hm2 = max(t l2, t r2)  (halo rows via DMA)
      m7  = max(hm2 up2, hm2 dn2, m5 l1, r1, up1, dn1)      (ellipse7)
    erosion mirrored with min. DVE is the only engine with elementwise
    max/min on this target (Pool/GPSIMD TensorTensor and DMA cce max fail
    the NeuronCore-v3 ISA checks), so every other engine carries the rest:
  - sum(p*m) on PE: "diagonal" matmuls (weights = 128-col blocks of p,
    moving = m blocks) accumulated into one [128,128] PSUM bank per
    quantity; the diagonal is extracted as soon as each chain completes
    (ACT copy to SBUF, DVE multiply by an identity input + row-reduce).
  - sum(m) on PE: ones-weight matmul chains through two alternating
    [1,512] PSUM banks, read out by ACT accum_out between chains. The
    last quantity instead rides ACT in row quarters so the tail stays
    short.
  - sum(p) rides the f32->f16 cast on ACT as accum_out.
  - m3/m7 rotate through 2-deep tile pools so a new side's folds never
    wait on the previous side's PE consumers; the last side's m7 folds
    and PE/ACT consumption are row-quartered to shrink the tail.
  - Each core writes 30 partial sums; the host combines into the scalar.
"""

import numpy as np

B, C_IN, H, W = 16, 1, 1024, 1024
NCORES = 8
BPC = B // NCORES      # images per core
P = 128                # SBUF partitions
R = H // P             # 8 slab rows per partition
EPS = 1e-7
CH = 2                 # slab rows per f32 staging chunk
NSUM = 18              # sums cols: 8 p-cast + 6 inter + 4 tail-msum
NOUT = 30              # 18 sums + 12 msum scalars

_CACHE = {}


def build_nc():
    import concourse.bacc as bacc
    import concourse.mybir as mybir
    import concourse.tile as tile

    f32 = mybir.dt.float32
    f16 = mybir.dt.float16
    MAX = mybir.AluOpType.max
    MIN = mybir.AluOpType.min
    ADD = mybir.AluOpType.add
    MULT = mybir.AluOpType.mult
    COPY = mybir.ActivationFunctionType.Copy

    C = W
    n_img = BPC
    TROWS = R + 4   # t/hm2: 2 halo rows above+below
    MROWS = R + 2   # m3/m5: 1 halo row above+below
    MC = C + 2      # m3/m5: 1 pad col each side (buf col = img col + 1)

    nc = bacc.Bacc("TRN2", target_bir_lowering=False)
    t_dram = nc.dram_tensor("teacher", [n_img, R * P, C], f32, kind="ExternalInput")
    p_dram = nc.dram_tensor("pred", [n_img, R * P, C], f32, kind="ExternalInput")
    i_dram = nc.dram_tensor("ident", [P, P], f32, kind="ExternalInput")
    out_dram = nc.dram_tensor("partials", [1, NOUT], f32, kind="ExternalOutput")

    with tile.TileContext(nc) as tc:
        with (
            tc.tile_pool(name="stage", bufs=2) as stage_pool,
            tc.tile_pool(name="img", bufs=2) as img_pool,
            tc.tile_pool(name="imgp", bufs=1) as imgp_pool,
            tc.tile_pool(name="morph2", bufs=2) as morph2_pool,
            tc.tile_pool(name="morph", bufs=1) as morph_pool,
            tc.tile_pool(name="small", bufs=1) as small_pool,
            tc.tile_pool(name="psum", bufs=1, space="PSUM") as psum_pool,
        ):
            sums = small_pool.tile([P, NSUM], f32, tag="sums")
            ones16 = small_pool.tile([P, 1], f16, tag="ones16")
            ones32 = small_pool.tile([P, 1], f32, tag="ones32")
            ident = small_pool.tile([P, P], f32, tag="ident")
            scr = small_pool.tile([P, P], f32, tag="scr")
            scr512 = small_pool.tile([1, 512], f32, tag="scr512")
            scr2 = small_pool.tile([P, P], f32, tag="scr2")
            msums1 = small_pool.tile([1, 12], f32, tag="msums1")
            outsb = small_pool.tile([1, NOUT], f32, tag="outsb")
            nc.vector.memset(sums[:], 0.0)
            nc.vector.memset(ones16[:], 1.0)
            nc.vector.memset(ones32[:], 1.0)
            nc.sync.dma_start(ident[:], i_dram[:])

            m5 = morph_pool.tile([P, MROWS, MC], f16, tag="m5")
            hm2 = morph_pool.tile([P, TROWS, C], f16, tag="hm2")

            # one PSUM bank per quantity holding the [128,128] diag tile,
            # plus a single shared [1,512] bank for the msum chains
            ps_q = [psum_pool.tile([P, P], f32, tag=f"psq{q}", name=f"psq{q}")
                    for q in range(6)]
            ps_ms = [psum_pool.tile([1, 512], f32, tag="psms0", name="psms0"),
                     psum_pool.tile([1, 512], f32, tag="psms1", name="psms1")]
            diag_cnt = [0] * 6
            iblk = [(c0, 128) for c0 in range(0, C, 128)]
            DIAG_TOT = n_img * R * len(iblk)

            def halo1(m):
                """1-row top/bottom halo fill (replicate at image edges)."""
                nc.sync.dma_start(m[1:P, 0:1, :], m[0:P - 1, R:R + 1, :])
                nc.sync.dma_start(m[0:P - 1, MROWS - 1:MROWS, :], m[1:P, 1:2, :])
                nc.sync.dma_start(m[0:1, 0:1, :], m[0:1, 1:2, :])
                nc.sync.dma_start(m[P - 1:P, MROWS - 1:MROWS, :],
                                  m[P - 1:P, MROWS - 2:MROWS - 1, :])

            def halo2(m):
                """2-row top/bottom halo fill for a TROWS buffer."""
                nc.sync.dma_start(m[1:P, 0:2, :], m[0:P - 1, R:R + 2, :])
                nc.sync.dma_start(m[0:P - 1, TROWS - 2:TROWS, :], m[1:P, 2:4, :])
                for hr in (0, 1):
                    nc.sync.dma_start(m[0:1, hr:hr + 1, :], m[0:1, 2:3, :])
                for hr in (TROWS - 2, TROWS - 1):
                    nc.sync.dma_start(m[P - 1:P, hr:hr + 1, :],
                                      m[P - 1:P, TROWS - 3:TROWS - 2, :])

            def pe_sums(q, img, p_t, buf, rofs, cofs, r0=0, r1=R,
                        msum_pe=True):
                """sum(p*m) on PE via diagonal-accumulate matmuls; sum(m) on
                PE via a ones-matmul chain through the shared msum bank,
                read out by ACT into msums1 before the next chain reuses it."""
                for r in range(r0, r1):
                    if msum_pe:
                        for c0 in (0, 512):
                            nc.tensor.matmul(
                                ps_ms[q % 2][:, 0:512], ones16[:],
                                buf[:, rofs + r, cofs + c0:cofs + c0 + 512],
                                start=(r == 0 and c0 == 0),
                                stop=(r == R - 1 and c0 == 512))
                    for (c0, cw) in iblk:
                        nc.tensor.matmul(
                            ps_q[q][0:cw, 0:cw], p_t[:, r, c0:c0 + cw],
                            buf[:, rofs + r, cofs + c0:cofs + c0 + cw],
                            start=(diag_cnt[q] == 0),
                            stop=(diag_cnt[q] == DIAG_TOT - 1))
                        diag_cnt[q] += 1
                if r1 < R:
                    return
                if msum_pe:
                    nc.scalar.activation(scr512[:], ps_ms[q % 2][:], COPY,
                                         accum_out=msums1[:, img * 6 + q:
                                                          img * 6 + q + 1])
                if diag_cnt[q] == DIAG_TOT:
                    # chain complete: extract the PSUM diagonal now, off the
                    # critical tail
                    nc.scalar.activation(scr[:], ps_q[q][:], COPY)
                    nc.vector.tensor_tensor(scr2[:], scr[:], ident[:], op=MULT)
                    nc.vector.tensor_reduce(sums[:, 8 + q:9 + q], scr2[:],
                                            axis=mybir.AxisListType.X, op=ADD)

            for img in range(n_img):
                t_view = t_dram[img].rearrange("(p r) w -> p r w", p=P)
                p_view = p_dram[img].rearrange("(p r) w -> p r w", p=P)
                t = img_pool.tile([P, TROWS, C], f16, tag="t", name="t")
                p_t = imgp_pool.tile([P, R, C], f16, tag="p", name="p")
                # t loads as gpsimd (SWDGE) casting DMAs: f32 HBM -> f16
                # SBUF in flight, no staging and no ACT cast on the
                # startup-critical path
                for (ra, rb) in ((0, 2), (2, 4), (4, 6), (6, 8)):
                    nc.gpsimd.dma_start(t[:, 2 + ra:2 + rb, :],
                                        t_view[:, ra:rb, :])
                halo2(t)
                for r0 in range(0, R, CH):
                    st = stage_pool.tile([P, CH, C], f32, tag="stage", name="stage")
                    nc.sync.dma_start(st[:], p_view[:, r0:r0 + CH, :])
                    nc.scalar.activation(p_t[:, r0:r0 + CH, :], st[:], COPY,
                                         accum_out=sums[:, img * 4 + r0 // CH:
                                                        img * 4 + r0 // CH + 1])

                for is_dil, base_q in ((True, 0), (False, 3)):
                    OP = MAX if is_dil else MIN
                    fill = -1e4 if is_dil else 1e4
                    V = nc.vector
                    m3 = morph2_pool.tile([P, MROWS, MC], f16, tag="m3",
                                          name="m3")
                    m7 = morph2_pool.tile([P, R, C], f16, tag="m7", name="m7")
                    first = img == 0 and is_dil
                    rsp = (((0, 2), (2, 4), (4, 6), (6, 8)) if first
                           else ((0, 8),))

                    V.memset(m3[:, :, 0:1], fill)
                    V.memset(m3[:, :, MC - 1:MC], fill)
                    V.memset(m5[:, :, 0:1], fill)
                    V.memset(m5[:, :, MC - 1:MC], fill)

                    def emit_h1(OP=OP, rsp=rsp):
                        # hmax3(t) built directly into m3 (no h1 buffer):
                        # m3 = op(t l1, t r1); then op= t center
                        for (ra, rb) in rsp:
                            V.tensor_tensor(m3[:, 1 + ra:1 + rb, 2:C],
                                            t[:, 2 + ra:2 + rb, 0:C - 2],
                                            t[:, 2 + ra:2 + rb, 2:C], op=OP)
                        V.tensor_tensor(m3[:, 1:9, 1:2], t[:, 2:10, 1:2],
                                        t[:, 2:10, 1:2], op=OP)
                        V.tensor_tensor(m3[:, 1:9, C:C + 1],
                                        t[:, 2:10, C - 2:C - 1],
                                        t[:, 2:10, C - 2:C - 1], op=OP)
                        for (ra, rb) in rsp:
                            V.tensor_tensor(m3[:, 1 + ra:1 + rb, 1:C + 1],
                                            m3[:, 1 + ra:1 + rb, 1:C + 1],
                                            t[:, 2 + ra:2 + rb, :], op=OP)

                    def emit_hm2(OP=OP, rsp=rsp):
                        # hm2 = op(t l2, t r2); halo rows via DMA. Emitted
                        # early so the halo DMAs hide behind later DVE work.
                        for (ra, rb) in rsp:
                            V.tensor_tensor(hm2[:, 2 + ra:2 + rb, 2:C - 2],
                                            t[:, 2 + ra:2 + rb, 0:C - 4],
                                            t[:, 2 + ra:2 + rb, 4:C], op=OP)
                        V.tensor_tensor(hm2[:, 2:10, 0:2], t[:, 2:10, 2:4],
                                        t[:, 2:10, 2:4], op=OP)
                        V.tensor_tensor(hm2[:, 2:10, C - 2:C],
                                        t[:, 2:10, C - 4:C - 2],
                                        t[:, 2:10, C - 4:C - 2], op=OP)
                        halo2(hm2)

                    if first:
                        emit_h1()
                        emit_hm2()
                    else:
                        emit_hm2()
                        emit_h1()

                    # ---- m3 = op(hmax3, t up1, t dn1) ----
                    V.tensor_tensor(m3[:, 1:9, 1:C + 1], m3[:, 1:9, 1:C + 1],
                                    t[:, 1:9, :], op=OP)
                    V.tensor_tensor(m3[:, 1:9, 1:C + 1], m3[:, 1:9, 1:C + 1],
                                    t[:, 3:11, :], op=OP)
                    halo1(m3)
                    pe_sums(base_q + 0, img, p_t, m3, 1, 1)

                    # ---- m7 init = op(hm2 up2, dn2) ----
                    V.tensor_tensor(m7[:], hm2[:, 0:8, :], hm2[:, 4:12, :], op=OP)

                    # ---- m5 = op(m3 l1, r1, up1, dn1) ----
                    V.tensor_tensor(m5[:, 1:9, 1:C + 1], m3[:, 1:9, 0:C],
                                    m3[:, 1:9, 2:C + 2], op=OP)
                    V.tensor_tensor(m5[:, 1:9, 1:C + 1], m5[:, 1:9, 1:C + 1],
                                    m3[:, 0:8, 1:C + 1], op=OP)
                    V.tensor_tensor(m5[:, 1:9, 1:C + 1], m5[:, 1:9, 1:C + 1],
                                    m3[:, 2:10, 1:C + 1], op=OP)
                    halo1(m5)
                    pe_sums(base_q + 1, img, p_t, m5, 1, 1)

                    # ---- m7 folds: op(m7, m5 l1, r1, up1, dn1) ----
                    # Last side: fully row-quartered so PE/ACT consumption
                    # pipelines with the folds and the tail shrinks.
                    last = img == n_img - 1 and not is_dil
                    if last:
                        for qi, (ra, rb) in enumerate(
                                ((0, 2), (2, 4), (4, 6), (6, 8))):
                            V.tensor_tensor(m7[:, ra:rb, :], m7[:, ra:rb, :],
                                            m5[:, 1 + ra:1 + rb, 0:C], op=OP)
                            V.tensor_tensor(m7[:, ra:rb, :], m7[:, ra:rb, :],
                                            m5[:, 1 + ra:1 + rb, 2:C + 2],
                                            op=OP)
                            V.tensor_tensor(m7[:, ra:rb, :], m7[:, ra:rb, :],
                                            m5[:, ra:rb, 1:C + 1], op=OP)
                            V.tensor_tensor(m7[:, ra:rb, :], m7[:, ra:rb, :],
                                            m5[:, 2 + ra:2 + rb, 1:C + 1],
                                            op=OP)
                            pe_sums(base_q + 2, img, p_t, m7, 0, 0,
                                    r0=ra, r1=rb, msum_pe=False)
                            nc.scalar.activation(
                                m7[:, ra:rb, :], m7[:, ra:rb, :], COPY,
                                accum_out=sums[:, 14 + qi:15 + qi])
                    else:
                        V.tensor_tensor(m7[:], m7[:], m5[:, 1:9, 0:C], op=OP)
                        V.tensor_tensor(m7[:], m7[:], m5[:, 1:9, 2:C + 2], op=OP)
                        V.tensor_tensor(m7[:], m7[:], m5[:, 0:8, 1:C + 1], op=OP)
                        V.tensor_tensor(m7[:], m7[:], m5[:, 2:10, 1:C + 1], op=OP)
                        pe_sums(base_q + 2, img, p_t, m7, 0, 0)

            # ---- epilogue ----
            nc.tensor.matmul(ps_ms[0][0:1, 0:NSUM], ones32[:], sums[:],
                             start=True, stop=True)
            nc.scalar.activation(outsb[:, 0:NSUM], ps_ms[0][0:1, 0:NSUM], COPY)
            nc.scalar.activation(outsb[:, NSUM:NSUM + 12], msums1[:], COPY)
            nc.sync.dma_start(out_dram[:], outsb[:])

    nc.compile()
    return nc


def combine_partials(partials):
    """partials: [ncores, 26] float32 -> scalar loss (mirrors reference)."""
    partials = np.asarray(partials, dtype=np.float64)
    p_sum = partials[:, 0:8].sum()
    total = 0.0
    for q in range(6):
        inter = partials[:, 8 + q].sum()
        msum = 0.0
        for img in range(BPC):
            if img == BPC - 1 and q == 5:
                msum += partials[:, 14:18].sum()
            else:
                msum += partials[:, 18 + img * 6 + q].sum()
        card = p_sum + msum
        score = 2.0 * inter / max(card, EPS)
        total += (1.0 - score) * (1.0 if msum > 0 else 0.0)
    return np.float32(total / 3.0)


def kernel(pred_student_prob, teacher_prob):
    from concourse.bass_utils import run_bass_kernel_spmd

    if "nc" not in _CACHE:
        _CACHE["nc"] = build_nc()
    nc = _CACHE["nc"]

    pred = np.ascontiguousarray(pred_student_prob.reshape(B, H, W), dtype=np.float32)
    teach = np.ascontiguousarray(teacher_prob.reshape(B, H, W), dtype=np.float32)
    ident = np.eye(P, dtype=np.float32)
    in_maps = []
    for c in range(NCORES):
        sl = slice(c * BPC, (c + 1) * BPC)
        in_maps.append({
            "teacher": np.ascontiguousarray(teach[sl]),
            "pred": np.ascontiguousarray(pred[sl]),
            "ident": ident,
        })
    res = run_bass_kernel_spmd(nc, in_maps, core_ids=list(range(NCORES)))
    partials = np.stack([res.results[c]["partials"][0] for c in range(NCORES)])
    return combine_partials(partials)
